# revision 33
# baseline (speedup 1.0000x reference)
"""Binarized-MLP (BinaryNet) forward on 8 Trainium2 NeuronCores.

Reference computation (per nn_FC_large):
    h = sign(x[:, :768]) @ sign(w1).T + b1 ; BN1 ; -> sign
    h = sign(h) @ sign(w2).T + b2         ; BN2 ; -> sign
    h = sign(h) @ sign(w3).T + b3         ; BN3 ; -> sign
    h = sign(h) @ sign(w4).T + b4         ; BN4 ; log_softmax

Strategy (data parallel, batch 16384 -> 2048 rows/core):
  * All matmul operands are exactly representable in fp8: weights are
    binarized host-side to {-1,+1}; activations are kept as a in {0,1}
    (a = [pre-act >= 0]) and the identity
        sign_mm = 2*(Wsign @ a) - rowsum(Wsign)
    folds rowsum into per-neuron thresholds, so each layer's epilogue is a
    single DVE is_ge producing the next layer's {0,1} fp8 activations.
  * Matmuls run in fp8e4 with perf_mode=DoubleRow (K=256 per instruction),
    activations stored feature-major [F, B] in SBUF across the whole net.
    The PE stream is the bottleneck (~216 ns per 512-col DR matmul, ~97% of
    the DoubleRow streaming roofline), so everything else hides under it.
  * Layer-1 activations are binarized on the HOST ([x >= 0] is exact in any
    dtype) and DMA'd straight into SBUF as fp8 {0,1}: halves the input DMA
    and removes the on-device binarize from the critical path. L1 starts
    ~4.5us into the kernel (vs ~20us with on-device bf16 binarize).
  * BatchNorm (eval) + bias fold into thresholds (layers 1-3) / an affine
    (layer 4). Layer-4 logits are PE-transposed to batch-major (interleaved
    with the L4 matmul groups) into a single PSUM bank, and log_softmax runs
    as 7 whole-tile ops (reduce_max / broadcast-sub / exp / reduce-add / ln /
    broadcast-sub) instead of 16 per-batch-tile chains.
  * The ACT table set `natural_log_exp_and_others` (identity+exp+ln) is
    preloaded at kernel start so no table swap lands on the critical tail.
  * Accumulation is exact: products are in {-1,0,1}, sums are integers
    well inside fp32, so the binary pipeline is bit-exact w.r.t. the
    reference up to threshold ties (probability ~0 with random BN stats).

Everything is hardcoded for x:[16384,784], layers 768->4096->4096->4096->10.
"""

import numpy as np
import ml_dtypes
from contextlib import ExitStack

import concourse.mybir as mybir
import concourse.tile as tile
from concourse import bacc
from concourse.bass_utils import run_bass_kernel_spmd
from concourse.masks import make_identity

FP32 = mybir.dt.float32
FP8 = mybir.dt.float8e4
NP_FP8 = ml_dtypes.float8_e4m3

EPS = 1e-5
B, IND, HID, OUT = 16384, 768, 4096, 10
N_CORES = 8
BC = B // N_CORES  # 2048 batch rows per core

# Knobs (test.py may flip TRACE before calling kernel()).
TRACE = False
TRACE_KWARGS = {}
LAST_RESULTS = None  # BassKernelResults of the most recent run

# act_info.json set 6 = natural_log_exp_and_others: {ln, exp, copy, identity}
ACT_SET_LN_EXP = 6


# --------------------------------------------------------------------------
# Device program
# --------------------------------------------------------------------------

def _layer_fwd(nc, wpool, psum_pool, act_in, C, wdr, thr_sb, act_out, Mt, bc,
               dma_engine=None, first_wt=None, pads=None):
    """One binarized layer: act_out = [W_fp8dr.T @ act_in >= thr] in {0,1} fp8.

    act_in : SBUF AP [128, C, 2, bc] fp8 ({0,1})
    wdr    : DRAM [Mt, 128, C, 2, 128] fp8 ({-1,+1})
    thr_sb : SBUF [128, Mt] fp32
    act_out: SBUF AP [128, Mt//2, 2, bc] fp8
    """
    NT = bc // 512
    DR = mybir.MatmulPerfMode.DoubleRow
    dma_engine = dma_engine or nc.sync
    first_wt = first_wt or {}
    for mt in range(Mt):
        if mt in first_wt:
            wt = first_wt[mt]
        else:
            wt = wpool.tile([128, C, 2, 128], FP8, tag="w")
            dma_engine.dma_start(out=wt[:], in_=wdr[mt])
        pss = [psum_pool.tile([128, 512], FP32, tag="psum", name=f"ps{mt}_{n}")
               for n in range(NT)]
        for c in range(C):
            for n in range(NT):
                nc.tensor.matmul(
                    pss[n][:],
                    lhsT=wt[:, c, :, :],
                    rhs=act_in[:, c, :, 512 * n:512 * (n + 1)],
                    start=(c == 0),
                    stop=(c == C - 1),
                    perf_mode=DR,
                )
            if pads and (mt, c) in pads:
                pads[(mt, c)]()
        for n in range(NT):
            nc.vector.tensor_scalar(
                out=act_out[:, mt // 2, mt % 2, 512 * n:512 * (n + 1)],
                in0=pss[n][:],
                scalar1=thr_sb[:, mt:mt + 1],
                scalar2=None,
                op0=mybir.AluOpType.is_ge,
            )


def build_program(bc=BC, dump_acts=False):
    """Build the per-core Bass/Tile program (SPMD; identical on all cores)."""
    NT = bc // 512
    BT = bc // 128
    DR = mybir.MatmulPerfMode.DoubleRow

    nc = bacc.Bacc(None, target_bir_lowering=False, debug=False)
    dbg = {}
    if dump_acts:
        for nm in ("act1d", "act2d", "act3d", "act4d"):
            cdim = 3 if nm == "act1d" else 16
            dbg[nm] = nc.dram_tensor(
                nm, [128, cdim, 2, bc], FP8, kind="ExternalOutput")
        dbg["h4d"] = nc.dram_tensor("h4d", [128, 512], FP32,
                                    kind="ExternalOutput")

    a1 = nc.dram_tensor("a1", [128, 3, 2, bc], FP8, kind="ExternalInput")
    w1 = nc.dram_tensor("w1dr", [32, 128, 3, 2, 128], FP8, kind="ExternalInput")
    w2 = nc.dram_tensor("w2dr", [32, 128, 16, 2, 128], FP8, kind="ExternalInput")
    w3 = nc.dram_tensor("w3dr", [32, 128, 16, 2, 128], FP8, kind="ExternalInput")
    w4 = nc.dram_tensor("w4dr", [128, 32, 16], FP8, kind="ExternalInput")
    thrs = nc.dram_tensor("thrs", [128, 3, 32], FP32, kind="ExternalInput")
    c4 = nc.dram_tensor("c4", [128, 2], FP32, kind="ExternalInput")
    out = nc.dram_tensor("out", [128, bc // 128, OUT], FP32,
                         kind="ExternalOutput")

    with tile.TileContext(nc) as tc, ExitStack() as ctx:
        consts = ctx.enter_context(tc.tile_pool(name="consts", bufs=1))
        a1pool = ctx.enter_context(tc.tile_pool(name="a1pool", bufs=1))
        apool = ctx.enter_context(
            tc.tile_pool(name="apool", bufs=3 if dump_acts else 2))
        wpool = ctx.enter_context(tc.tile_pool(name="wpool", bufs=4))
        smpool = ctx.enter_context(tc.tile_pool(name="smpool", bufs=3))
        psum_pool = ctx.enter_context(
            tc.tile_pool(name="psum", bufs=8, space="PSUM"))

        # Preload the exp+ln+identity ACT table set (runs during the ~7us
        # framework preamble, overlapping the DMA queue) so neither the L4
        # affine nor the softmax tail pays a ~2.7us table swap.
        nc.scalar.add_instruction(mybir.InstLoadActFuncSet(
            name=nc.get_next_instruction_name(), ins=[], outs=[],
            act_func_set_id=ACT_SET_LN_EXP))

        thrs_sb = consts.tile([128, 3, 32], FP32, tag="thrs")
        c4_sb = consts.tile([128, 2], FP32, tag="c4")
        w4_sb = consts.tile([128, 32, 16], FP8, tag="w4")
        ident = consts.tile([128, 128], FP32, tag="ident")
        # one logits tile per batch-slice: the dep tracker keys on byte
        # ranges (not partitions), so a single shared tile would falsely
        # serialize the four parallel affines across engines
        h4g = [consts.tile([128, 512], FP32, tag="h4", name=f"h4g{g}")
               for g in range(4)]
        out_sb = consts.tile([128, BT, OUT], FP32, tag="outsb")
        thr1_sb = thrs_sb[:, 0, :]
        thr2_sb = thrs_sb[:, 1, :]
        thr3_sb = thrs_sb[:, 2, :]

        # ---- layer-1 activations: host-binarized {0,1} fp8. The first
        # k-chunk and the first L1 weight tile gate the start of the MM
        # stream, so they go FIRST on the sync queue (4KB/768B lines) with
        # everything else (small-line const DMAs, remaining chunks) behind
        # or on the scalar queue, keeping the DMA engines clear for them.
        # The early DMA burst is delivery-bandwidth-bound (~200-250 GB/s
        # effective across the two HWDGE queues), so everything is issued in
        # strict NEED order, with each act1 k-chunk split across both queues
        # so it lands at aggregate bandwidth: c0 -> w1[0] -> c1 -> c2 ->
        # w1[1] -> w1[2]. Later w1 tiles stream on sync far ahead of their
        # ~2.6us/tile consumption.
        act1 = a1pool.tile([128, 3, 2, bc], FP8, tag="act1")
        w1t = {}

        def _w1t(mt, eng):
            w1t[mt] = wpool.tile([128, 3, 2, 128], FP8, tag="w",
                                 name=f"w1t{mt}")
            eng.dma_start(out=w1t[mt][:], in_=w1[mt])

        nc.sync.dma_start(out=act1[:, 0], in_=a1[:, 0])
        _w1t(0, nc.sync)
        nc.sync.dma_start(out=act1[:, 1], in_=a1[:, 1])
        _w1t(1, nc.sync)
        _w1t(2, nc.sync)
        nc.sync.dma_start(out=act1[:, 2], in_=a1[:, 2])

        # thrs on the scalar ring (thr1 first needed ~20us in); c4/w4 are
        # only needed by layer 4 (~990us) and go at the very back.
        nc.scalar.dma_start(out=thrs_sb[:], in_=thrs[:])

        # PE warm-up: short (N=128) garbage DR matmuls (memset operands,
        # never-read psum) bridge the ~3.5us between the earliest possible
        # PE activity (~7.9us, post-preamble) and act1-c0/w1t0 landing
        # (~10.5us) at ~107ns granularity, keeping the HAM busy window
        # counting continuously so the clock un-gates ~3.4us after the
        # first warmup, right as layer 1 opens.
        warm = consts.tile([128, 2, 128], FP8, tag="warm")
        nc.gpsimd.memset(warm[:], 0.0)
        wps = psum_pool.tile([128, 128], FP32, tag="psum", name="warmps")
        for _ in range(34):
            nc.tensor.matmul(
                wps[:], lhsT=warm[:], rhs=warm[:],
                start=True, stop=True, perf_mode=DR)
        make_identity(nc, ident[:])

        def _pad(n_mms):
            # garbage matmuls bridging a known data stall so the PE never
            # idles long enough for the HAM clock gate to re-throttle
            def f():
                for _ in range(n_mms):
                    nc.tensor.matmul(
                        wps[:], lhsT=warm[:], rhs=warm[:],
                        start=True, stop=True, perf_mode=DR)
            return f

        # ---- layers 1-3 ----
        # a1-c2 lands ~2us after layer 1 first consumes it (delivery-bound);
        # pad the (mt0, c1->c2) boundary so the clock stays un-gated.
        act2 = apool.tile([128, 16, 2, bc], FP8, tag="actbig")
        _layer_fwd(nc, wpool, psum_pool, act1, 3, w1, thr1_sb, act2, 32, bc,
                   dma_engine=nc.scalar, first_wt=w1t,
                   pads={(0, 1): _pad(24)})

        # L4 consts ride the scalar ring behind the last L1 weight tiles
        nc.scalar.dma_start(out=c4_sb[:], in_=c4[:])
        nc.scalar.dma_start(out=w4_sb[:], in_=w4[:])


        act3 = apool.tile([128, 16, 2, bc], FP8, tag="actbig")
        _layer_fwd(nc, wpool, psum_pool, act2, 16, w2, thr2_sb, act3, 32, bc)
        act4 = apool.tile([128, 16, 2, bc], FP8, tag="actbig")
        _layer_fwd(nc, wpool, psum_pool, act3, 16, w3, thr3_sb, act4, 32, bc)

        # ---- layer 4: logits (M padded 10->16), affine folds BN+rowsum.
        # M=16 would waste 7/8 of the PE array in the normal layout, so the
        # four 512-column batch slices run CONCURRENTLY on the four 32-wide
        # PE column groups (plain fp8, no DoubleRow -- DR and column tiling
        # are mutually exclusive on the XBUS budget): slice g accumulates
        # K=4096 as 32 k-chunks into PSUM partitions [32g, 32g+16). The
        # whole GEMM spans ~32 matmul slots instead of 64 DR slots.
        # Affines (scalar engine) land the logits in SBUF at the same
        # partition bases; transposes lift each 128-batch tile to
        # batch-major [128, 10] in one PSUM bank for the softmax chain.
        tps = psum_pool.tile([128, BT * OUT], FP32, tag="psum", name="tps")
        # one PSUM bank per column group: a shared bank would serialize the
        # four affine readers (same-bank read-port hazard in the framework)
        ps4g = [psum_pool.tile([128, 512], FP32, tag="psum", name=f"ps4g{g}")
                for g in range(NT)]
        for kc in range(32):
            for g in range(NT):
                nc.tensor.matmul(
                    ps4g[g][32 * g:32 * g + 16, :],
                    lhsT=w4_sb[:, kc, :],
                    rhs=act4[:, kc // 2, kc % 2, 512 * g:512 * (g + 1)],
                    start=(kc == 0),
                    stop=(kc == 31),
                    tile_position=(0, 32 * g),
                )
        # Affines alternate ACT/DVE so all four land in ~1.5us instead of
        # 3.1us of serial ACT; the transposes (~60ns pipelined spacing each)
        # then drain the whole tail quickly.
        for g in range(NT):
            sc = c4_sb[32 * g:32 * g + 16, 0:1]
            bi = c4_sb[32 * g:32 * g + 16, 1:2]
            src = ps4g[g][32 * g:32 * g + 16, :]
            dst = h4g[g][32 * g:32 * g + 16, :]
            if g % 2 == 0:
                nc.scalar.activation(
                    dst, src, mybir.ActivationFunctionType.Identity,
                    bias=bi, scale=sc)
            else:
                nc.vector.tensor_scalar(
                    out=dst, in0=src, scalar1=sc, scalar2=bi,
                    op0=mybir.AluOpType.mult, op1=mybir.AluOpType.add)
        for g in range(NT):
            for b in range(4):
                bt = 4 * g + b
                nc.tensor.transpose(
                    tps[:, OUT * bt:OUT * (bt + 1)],
                    h4g[g][32 * g:32 * g + OUT, 128 * b:128 * (b + 1)],
                    ident[32 * g:32 * g + OUT, 32 * g:32 * g + OUT],
                    tile_position=(32 * g, 0))

        # ---- log_softmax, batched over all 16 batch-tiles at once ----
        tps3 = tps[:].rearrange("p (b o) -> p b o", b=BT)
        mx = smpool.tile([128, BT], FP32, tag="mx", bufs=1)
        nc.vector.reduce_max(mx[:], tps3, axis=mybir.AxisListType.X)
        sh = smpool.tile([128, BT, OUT], FP32, tag="sh", bufs=1)
        nc.vector.scalar_tensor_tensor(
            out=sh[:], in0=tps3, scalar=0.0,
            in1=mx[:].unsqueeze(2).broadcast_to([128, BT, OUT]),
            op0=mybir.AluOpType.add, op1=mybir.AluOpType.subtract)
        ex = smpool.tile([128, BT, OUT], FP32, tag="ex", bufs=1)
        nc.scalar.activation(ex[:], sh[:], mybir.ActivationFunctionType.Exp)
        se = smpool.tile([128, BT], FP32, tag="se", bufs=1)
        nc.vector.reduce_sum(se[:], ex[:], axis=mybir.AxisListType.X)
        ls = smpool.tile([128, BT], FP32, tag="ls", bufs=1)
        nc.scalar.activation(ls[:], se[:], mybir.ActivationFunctionType.Ln)
        nc.vector.scalar_tensor_tensor(
            out=out_sb[:], in0=sh[:], scalar=0.0,
            in1=ls[:].unsqueeze(2).broadcast_to([128, BT, OUT]),
            op0=mybir.AluOpType.add, op1=mybir.AluOpType.subtract)

        # out dram is [128, BT, OUT] (partition-major, fully contiguous DMA);
        # the host reassembles batch order with a free transpose.
        nc.sync.dma_start(out=out[:], in_=out_sb[:])

        if dump_acts:
            nc.sync.dma_start(out=dbg["act1d"][:], in_=act1[:])
            nc.sync.dma_start(out=dbg["act2d"][:], in_=act2[:])
            nc.sync.dma_start(out=dbg["act3d"][:], in_=act3[:])
            nc.sync.dma_start(out=dbg["act4d"][:], in_=act4[:])
            nc.sync.dma_start(out=dbg["h4d"][:], in_=h4g[0][:])

    nc.compile()
    return nc


# --------------------------------------------------------------------------
# Host-side preparation
# --------------------------------------------------------------------------

def _pack_w_dr(ws_t):
    """[Fin, Fout] {-1,+1} -> [Mt, 128, C, 2, 128] fp8 DoubleRow layout.

    wdr[mt, ki, c, ko, mi] = ws_t[256*c + 128*ko + ki, 128*mt + mi]
    """
    fin, fout = ws_t.shape
    C, Mt = fin // 256, fout // 128
    w = ws_t.reshape(C, 2, 128, Mt, 128).transpose(3, 2, 0, 1, 4)
    return np.ascontiguousarray(w).astype(NP_FP8)


def prepare_consts(inputs):
    """Fold sign(w), BN, bias and the 0/1-activation rowsum correction.

    The device computes, per layer, a_dev = [mmA~ >= thr] where
    mmA~ = W~sign @ a_dev_prev over {0,1} activations. Negative BN scales
    (alpha <= 0) are handled exactly by tracking a per-neuron flip bit
    (a_true = 1 - a_dev) that folds into the *next* layer's weight signs:
    with s~ = s * (1-2*flip_in), mm_full = 2*(s~ @ a_dev) - rowsum(s~)
    holds for any flip pattern. Thresholds use integer snapping (mmA is
    always an integer), making the device comparison tie-free/exact.
    """
    consts = {}
    flip_in = np.zeros(IND)  # input layer: a_dev = [x >= 0] = ste_sign, exact
    for i in (1, 2, 3, 4):
        w = np.asarray(inputs[f"w{i}"]).astype(np.float64)
        b = np.asarray(inputs[f"b{i}"]).astype(np.float64)
        g = np.asarray(inputs[f"g{i}"]).astype(np.float64)
        be = np.asarray(inputs[f"be{i}"]).astype(np.float64)
        m = np.asarray(inputs[f"m{i}"]).astype(np.float64)
        v = np.asarray(inputs[f"v{i}"]).astype(np.float64)
        ws = np.where(w >= 0, 1.0, -1.0) * (1.0 - 2.0 * flip_in)  # [fo, fi]
        rowsum = ws.sum(axis=1)                                   # [fo]
        alpha = g / np.sqrt(v + EPS)
        if i < 4:
            # BN(mm_full + b) >= 0 with mm_full = 2*mmA - rowsum:
            #   alpha > 0:  a_true = [mmA >= u],  u = (m-b-be/a+rowsum)/2
            #   alpha < 0:  a_true = [mmA <= u] = 1 - [mmA >= floor(u)+1]
            #   alpha == 0: BN = be, constant sign
            u = (m - b - be / alpha_safe(alpha) + rowsum) / 2.0
            pos = alpha > 0
            thr = np.where(pos, np.ceil(u), np.floor(u) + 1.0)
            zero = alpha == 0
            if zero.any():
                # constant: a_true = [be >= 0]; force a_dev accordingly
                thr = np.where(zero & (be >= 0), -1e30, thr)
                thr = np.where(zero & (be < 0), 1e30, thr)
                pos = pos | zero
            flip_in = (~pos).astype(np.float64)
            consts.setdefault("_thrs", []).append(
                thr.reshape(32, 128).T.astype(np.float32))
            consts[f"w{i}dr"] = _pack_w_dr(ws.T)
        else:
            # logits = mmA*(2*alpha) + ((b - m - rowsum)*alpha + be).
            # c4 affine replicated at the four column-group partition bases
            # (batch slice g computes at PSUM/SBUF partitions [32g, 32g+16)).
            scale = 2.0 * alpha
            beta = (b - m - rowsum) * alpha + be
            c4 = np.zeros((128, 2), np.float32)
            for g4 in range(4):
                c4[32 * g4:32 * g4 + 10, 0] = scale.astype(np.float32)
                c4[32 * g4:32 * g4 + 10, 1] = beta.astype(np.float32)
            consts["c4"] = c4
            ws_t_pad = np.zeros((HID, 16), np.float64)
            ws_t_pad[:, :10] = ws.T
            # w4dr[ki, kc, m] = ws_t_pad[128*kc + ki, m]  (no DoubleRow)
            w4 = ws_t_pad.reshape(32, 128, 16).transpose(1, 0, 2)
            consts["w4dr"] = np.ascontiguousarray(w4).astype(NP_FP8)
    consts["thrs"] = np.ascontiguousarray(
        np.stack(consts.pop("_thrs"), axis=1))  # [128, 3, 32]
    return consts


def alpha_safe(a):
    return np.where(a == 0, 1.0, a)


_PROG_CACHE = {}


def _get_program(bc=BC):
    if bc not in _PROG_CACHE:
        _PROG_CACHE[bc] = build_program(bc)
    return _PROG_CACHE[bc]


def kernel(**inputs):
    global LAST_RESULTS
    x = np.asarray(inputs["x"], np.float32)
    assert x.shape == (B, 784)
    consts = prepare_consts(inputs)
    # Binarize on the host: [x >= 0] in {0,1} is exactly ste_sign's predicate
    # (-0.0 >= 0 is True), stored fp8 in the device's [128, 3, 2, B] k-chunk
    # layout (k = 256*c + 128*ko + p).
    a = (x[:, :IND] >= 0).astype(NP_FP8)                       # [B, 768]
    a1_full = np.ascontiguousarray(
        a.T.reshape(3, 2, 128, B).transpose(2, 0, 1, 3))       # [128,3,2,B]

    nc = _get_program(BC)
    in_maps = []
    for c in range(N_CORES):
        m = {"a1": np.ascontiguousarray(a1_full[:, :, :, c * BC:(c + 1) * BC])}
        m.update(consts)
        in_maps.append(m)

    res = run_bass_kernel_spmd(
        nc, in_maps, core_ids=list(range(N_CORES)), trace=TRACE,
        **TRACE_KWARGS)
    LAST_RESULTS = res
    # device out is [128, BT, 10] partition-major; restore batch order
    outs = [np.ascontiguousarray(r["out"].transpose(1, 0, 2).reshape(BC, OUT))
            for r in res.results]
    return np.concatenate(outs, axis=0)


# revision 34
# speedup vs baseline: 1.0001x; 1.0001x over previous
"""Binarized-MLP (BinaryNet) forward on 8 Trainium2 NeuronCores.

Reference computation (per nn_FC_large):
    h = sign(x[:, :768]) @ sign(w1).T + b1 ; BN1 ; -> sign
    h = sign(h) @ sign(w2).T + b2         ; BN2 ; -> sign
    h = sign(h) @ sign(w3).T + b3         ; BN3 ; -> sign
    h = sign(h) @ sign(w4).T + b4         ; BN4 ; log_softmax

Strategy (data parallel, batch 16384 -> 2048 rows/core):
  * All matmul operands are exactly representable in fp8: weights are
    binarized host-side to {-1,+1}; activations are kept as a in {0,1}
    (a = [pre-act >= 0]) and the identity
        sign_mm = 2*(Wsign @ a) - rowsum(Wsign)
    folds rowsum into per-neuron thresholds, so each layer's epilogue is a
    single DVE is_ge producing the next layer's {0,1} fp8 activations.
  * Matmuls run in fp8e4 with perf_mode=DoubleRow (K=256 per instruction),
    activations stored feature-major [F, B] in SBUF across the whole net.
    The PE stream is the bottleneck (~216 ns per 512-col DR matmul, ~97% of
    the DoubleRow streaming roofline), so everything else hides under it.
  * Layer-1 activations are binarized on the HOST ([x >= 0] is exact in any
    dtype) and DMA'd straight into SBUF as fp8 {0,1}: halves the input DMA
    and removes the on-device binarize from the critical path. L1 starts
    ~4.5us into the kernel (vs ~20us with on-device bf16 binarize).
  * BatchNorm (eval) + bias fold into thresholds (layers 1-3) / an affine
    (layer 4). Layer-4 logits are PE-transposed to batch-major (interleaved
    with the L4 matmul groups) into a single PSUM bank, and log_softmax runs
    as 7 whole-tile ops (reduce_max / broadcast-sub / exp / reduce-add / ln /
    broadcast-sub) instead of 16 per-batch-tile chains.
  * The ACT table set `natural_log_exp_and_others` (identity+exp+ln) is
    preloaded at kernel start so no table swap lands on the critical tail.
  * Accumulation is exact: products are in {-1,0,1}, sums are integers
    well inside fp32, so the binary pipeline is bit-exact w.r.t. the
    reference up to threshold ties (probability ~0 with random BN stats).

Everything is hardcoded for x:[16384,784], layers 768->4096->4096->4096->10.
"""

import numpy as np
import ml_dtypes
from contextlib import ExitStack

import concourse.mybir as mybir
import concourse.tile as tile
from concourse import bacc
from concourse.bass_utils import run_bass_kernel_spmd
from concourse.masks import make_identity

FP32 = mybir.dt.float32
FP8 = mybir.dt.float8e4
NP_FP8 = ml_dtypes.float8_e4m3

EPS = 1e-5
B, IND, HID, OUT = 16384, 768, 4096, 10
N_CORES = 8
BC = B // N_CORES  # 2048 batch rows per core

# Knobs (test.py may flip TRACE before calling kernel()).
TRACE = False
TRACE_KWARGS = {}
LAST_RESULTS = None  # BassKernelResults of the most recent run

# act_info.json set 6 = natural_log_exp_and_others: {ln, exp, copy, identity}
ACT_SET_LN_EXP = 6


# --------------------------------------------------------------------------
# Device program
# --------------------------------------------------------------------------

def _layer_fwd(nc, wpool, psum_pool, act_in, C, wdr, thr_sb, act_out, Mt, bc,
               dma_engine=None, first_wt=None, pads=None):
    """One binarized layer: act_out = [W_fp8dr.T @ act_in >= thr] in {0,1} fp8.

    act_in : SBUF AP [128, C, 2, bc] fp8 ({0,1})
    wdr    : DRAM [Mt, 128, C, 2, 128] fp8 ({-1,+1})
    thr_sb : SBUF [128, Mt] fp32
    act_out: SBUF AP [128, Mt//2, 2, bc] fp8
    """
    NT = bc // 512
    DR = mybir.MatmulPerfMode.DoubleRow
    dma_engine = dma_engine or nc.sync
    first_wt = first_wt or {}
    for mt in range(Mt):
        if mt in first_wt:
            wt = first_wt[mt]
        else:
            wt = wpool.tile([128, C, 2, 128], FP8, tag="w")
            dma_engine.dma_start(out=wt[:], in_=wdr[mt])
        pss = [psum_pool.tile([128, 512], FP32, tag="psum", name=f"ps{mt}_{n}")
               for n in range(NT)]
        for c in range(C):
            for n in range(NT):
                nc.tensor.matmul(
                    pss[n][:],
                    lhsT=wt[:, c, :, :],
                    rhs=act_in[:, c, :, 512 * n:512 * (n + 1)],
                    start=(c == 0),
                    stop=(c == C - 1),
                    perf_mode=DR,
                )
            if pads and (mt, c) in pads:
                pads[(mt, c)]()
        for n in range(NT):
            nc.vector.tensor_scalar(
                out=act_out[:, mt // 2, mt % 2, 512 * n:512 * (n + 1)],
                in0=pss[n][:],
                scalar1=thr_sb[:, mt:mt + 1],
                scalar2=None,
                op0=mybir.AluOpType.is_ge,
            )


def build_program(bc=BC, dump_acts=False):
    """Build the per-core Bass/Tile program (SPMD; identical on all cores)."""
    NT = bc // 512
    BT = bc // 128
    DR = mybir.MatmulPerfMode.DoubleRow

    nc = bacc.Bacc(None, target_bir_lowering=False, debug=False)
    dbg = {}
    if dump_acts:
        for nm in ("act1d", "act2d", "act3d", "act4d"):
            cdim = 3 if nm == "act1d" else 16
            dbg[nm] = nc.dram_tensor(
                nm, [128, cdim, 2, bc], FP8, kind="ExternalOutput")
        dbg["h4d"] = nc.dram_tensor("h4d", [128, 512], FP32,
                                    kind="ExternalOutput")

    a1 = nc.dram_tensor("a1", [128, 3, 2, bc], FP8, kind="ExternalInput")
    w1 = nc.dram_tensor("w1dr", [32, 128, 3, 2, 128], FP8, kind="ExternalInput")
    w2 = nc.dram_tensor("w2dr", [32, 128, 16, 2, 128], FP8, kind="ExternalInput")
    w3 = nc.dram_tensor("w3dr", [32, 128, 16, 2, 128], FP8, kind="ExternalInput")
    w4 = nc.dram_tensor("w4dr", [128, 32, 16], FP8, kind="ExternalInput")
    thrs = nc.dram_tensor("thrs", [128, 3, 32], FP32, kind="ExternalInput")
    c4 = nc.dram_tensor("c4", [128, 2], FP32, kind="ExternalInput")
    out = nc.dram_tensor("out", [128, bc // 128, OUT], FP32,
                         kind="ExternalOutput")

    with tile.TileContext(nc) as tc, ExitStack() as ctx:
        consts = ctx.enter_context(tc.tile_pool(name="consts", bufs=1))
        a1pool = ctx.enter_context(tc.tile_pool(name="a1pool", bufs=1))
        apool = ctx.enter_context(
            tc.tile_pool(name="apool", bufs=3 if dump_acts else 2))
        wpool = ctx.enter_context(tc.tile_pool(name="wpool", bufs=4))
        smpool = ctx.enter_context(tc.tile_pool(name="smpool", bufs=3))
        psum_pool = ctx.enter_context(
            tc.tile_pool(name="psum", bufs=8, space="PSUM"))

        # Preload the exp+ln+identity ACT table set (runs during the ~7us
        # framework preamble, overlapping the DMA queue) so neither the L4
        # affine nor the softmax tail pays a ~2.7us table swap.
        nc.scalar.add_instruction(mybir.InstLoadActFuncSet(
            name=nc.get_next_instruction_name(), ins=[], outs=[],
            act_func_set_id=ACT_SET_LN_EXP))

        thrs_sb = consts.tile([128, 3, 32], FP32, tag="thrs")
        c4_sb = consts.tile([128, 2], FP32, tag="c4")
        w4_sb = consts.tile([128, 32, 16], FP8, tag="w4")
        ident = consts.tile([128, 128], FP32, tag="ident")
        # one logits tile per batch-slice: the dep tracker keys on byte
        # ranges (not partitions), so a single shared tile would falsely
        # serialize the four parallel affines across engines
        h4g = [consts.tile([128, 512], FP32, tag="h4", name=f"h4g{g}")
               for g in range(4)]
        out_sb = consts.tile([128, BT, OUT], FP32, tag="outsb")
        thr1_sb = thrs_sb[:, 0, :]
        thr2_sb = thrs_sb[:, 1, :]
        thr3_sb = thrs_sb[:, 2, :]

        # ---- layer-1 activations: host-binarized {0,1} fp8. The first
        # k-chunk and the first L1 weight tile gate the start of the MM
        # stream, so they go FIRST on the sync queue (4KB/768B lines) with
        # everything else (small-line const DMAs, remaining chunks) behind
        # or on the scalar queue, keeping the DMA engines clear for them.
        # The early DMA burst is delivery-bandwidth-bound (~200-250 GB/s
        # effective across the two HWDGE queues), so everything is issued in
        # strict NEED order, with each act1 k-chunk split across both queues
        # so it lands at aggregate bandwidth: c0 -> w1[0] -> c1 -> c2 ->
        # w1[1] -> w1[2]. Later w1 tiles stream on sync far ahead of their
        # ~2.6us/tile consumption.
        act1 = a1pool.tile([128, 3, 2, bc], FP8, tag="act1")
        w1t = {}

        def _w1t(mt, eng):
            w1t[mt] = wpool.tile([128, 3, 2, 128], FP8, tag="w",
                                 name=f"w1t{mt}")
            eng.dma_start(out=w1t[mt][:], in_=w1[mt])

        nc.sync.dma_start(out=act1[:, 0], in_=a1[:, 0])
        _w1t(0, nc.sync)
        nc.sync.dma_start(out=act1[:, 1], in_=a1[:, 1])
        _w1t(1, nc.sync)
        _w1t(2, nc.sync)
        nc.sync.dma_start(out=act1[:, 2], in_=a1[:, 2])

        # thrs on the scalar ring (thr1 first needed ~20us in); c4/w4 are
        # only needed by layer 4 (~990us) and go at the very back.
        nc.scalar.dma_start(out=thrs_sb[:], in_=thrs[:])

        # PE warm-up: short (N=128) garbage DR matmuls (memset operands,
        # never-read psum) bridge the ~3.5us between the earliest possible
        # PE activity (~7.9us, post-preamble) and act1-c0/w1t0 landing
        # (~10.5us) at ~107ns granularity, keeping the HAM busy window
        # counting continuously so the clock un-gates ~3.4us after the
        # first warmup, right as layer 1 opens.
        warm = consts.tile([128, 2, 128], FP8, tag="warm")
        nc.gpsimd.memset(warm[:], 0.0)
        wps = psum_pool.tile([128, 128], FP32, tag="psum", name="warmps")
        for _ in range(34):
            nc.tensor.matmul(
                wps[:], lhsT=warm[:], rhs=warm[:],
                start=True, stop=True, perf_mode=DR)
        make_identity(nc, ident[:])

        def _pad(n_mms):
            # garbage matmuls bridging a known data stall so the PE never
            # idles long enough for the HAM clock gate to re-throttle
            def f():
                for _ in range(n_mms):
                    nc.tensor.matmul(
                        wps[:], lhsT=warm[:], rhs=warm[:],
                        start=True, stop=True, perf_mode=DR)
            return f

        # ---- layers 1-3 ----
        # a1-c2 lands ~2us after layer 1 first consumes it (delivery-bound);
        # pad the (mt0, c1->c2) boundary so the clock stays un-gated.
        act2 = apool.tile([128, 16, 2, bc], FP8, tag="actbig")
        _layer_fwd(nc, wpool, psum_pool, act1, 3, w1, thr1_sb, act2, 32, bc,
                   dma_engine=nc.scalar, first_wt=w1t,
                   pads={(0, 1): _pad(24)})

        # L4 consts: c4 must ride a HWDGE queue (a tiny engine-executed
        # DMACopy on the ACT queue would alias its completion into the ACT
        # op-count semaphore and falsely serialize the DVE affines behind
        # the ACT affines); w4 rides scalar behind the last L1 weight tiles.
        nc.sync.dma_start(out=c4_sb[:], in_=c4[:])
        nc.scalar.dma_start(out=w4_sb[:], in_=w4[:])


        act3 = apool.tile([128, 16, 2, bc], FP8, tag="actbig")
        _layer_fwd(nc, wpool, psum_pool, act2, 16, w2, thr2_sb, act3, 32, bc)
        act4 = apool.tile([128, 16, 2, bc], FP8, tag="actbig")
        _layer_fwd(nc, wpool, psum_pool, act3, 16, w3, thr3_sb, act4, 32, bc)

        # ---- layer 4: logits (M padded 10->16), affine folds BN+rowsum.
        # M=16 would waste 7/8 of the PE array in the normal layout, so the
        # four 512-column batch slices run CONCURRENTLY on the four 32-wide
        # PE column groups (plain fp8, no DoubleRow -- DR and column tiling
        # are mutually exclusive on the XBUS budget): slice g accumulates
        # K=4096 as 32 k-chunks into PSUM partitions [32g, 32g+16). The
        # whole GEMM spans ~32 matmul slots instead of 64 DR slots.
        # Affines (scalar engine) land the logits in SBUF at the same
        # partition bases; transposes lift each 128-batch tile to
        # batch-major [128, 10] in one PSUM bank for the softmax chain.
        tps = psum_pool.tile([128, BT * OUT], FP32, tag="psum", name="tps")
        # one PSUM bank per column group: a shared bank would serialize the
        # four affine readers (same-bank read-port hazard in the framework)
        ps4g = [psum_pool.tile([128, 512], FP32, tag="psum", name=f"ps4g{g}")
                for g in range(NT)]
        for kc in range(32):
            for g in range(NT):
                nc.tensor.matmul(
                    ps4g[g][32 * g:32 * g + 16, :],
                    lhsT=w4_sb[:, kc, :],
                    rhs=act4[:, kc // 2, kc % 2, 512 * g:512 * (g + 1)],
                    start=(kc == 0),
                    stop=(kc == 31),
                    tile_position=(0, 32 * g),
                )
        # Affines alternate ACT/DVE so all four land in ~1.5us instead of
        # 3.1us of serial ACT; the transposes (~60ns pipelined spacing each)
        # then drain the whole tail quickly.
        for g in range(NT):
            sc = c4_sb[32 * g:32 * g + 16, 0:1]
            bi = c4_sb[32 * g:32 * g + 16, 1:2]
            src = ps4g[g][32 * g:32 * g + 16, :]
            dst = h4g[g][32 * g:32 * g + 16, :]
            if g % 2 == 0:
                nc.scalar.activation(
                    dst, src, mybir.ActivationFunctionType.Identity,
                    bias=bi, scale=sc)
            else:
                nc.vector.tensor_scalar(
                    out=dst, in0=src, scalar1=sc, scalar2=bi,
                    op0=mybir.AluOpType.mult, op1=mybir.AluOpType.add)
        for g in range(NT):
            for b in range(4):
                bt = 4 * g + b
                nc.tensor.transpose(
                    tps[:, OUT * bt:OUT * (bt + 1)],
                    h4g[g][32 * g:32 * g + OUT, 128 * b:128 * (b + 1)],
                    ident[32 * g:32 * g + OUT, 32 * g:32 * g + OUT],
                    tile_position=(32 * g, 0))

        # ---- log_softmax, batched over all 16 batch-tiles at once ----
        tps3 = tps[:].rearrange("p (b o) -> p b o", b=BT)
        mx = smpool.tile([128, BT], FP32, tag="mx", bufs=1)
        nc.vector.reduce_max(mx[:], tps3, axis=mybir.AxisListType.X)
        sh = smpool.tile([128, BT, OUT], FP32, tag="sh", bufs=1)
        nc.vector.scalar_tensor_tensor(
            out=sh[:], in0=tps3, scalar=0.0,
            in1=mx[:].unsqueeze(2).broadcast_to([128, BT, OUT]),
            op0=mybir.AluOpType.add, op1=mybir.AluOpType.subtract)
        ex = smpool.tile([128, BT, OUT], FP32, tag="ex", bufs=1)
        nc.scalar.activation(ex[:], sh[:], mybir.ActivationFunctionType.Exp)
        se = smpool.tile([128, BT], FP32, tag="se", bufs=1)
        nc.vector.reduce_sum(se[:], ex[:], axis=mybir.AxisListType.X)
        ls = smpool.tile([128, BT], FP32, tag="ls", bufs=1)
        nc.scalar.activation(ls[:], se[:], mybir.ActivationFunctionType.Ln)
        nc.vector.scalar_tensor_tensor(
            out=out_sb[:], in0=sh[:], scalar=0.0,
            in1=ls[:].unsqueeze(2).broadcast_to([128, BT, OUT]),
            op0=mybir.AluOpType.add, op1=mybir.AluOpType.subtract)

        # out dram is [128, BT, OUT] (partition-major, fully contiguous DMA);
        # the host reassembles batch order with a free transpose.
        nc.sync.dma_start(out=out[:], in_=out_sb[:])

        if dump_acts:
            nc.sync.dma_start(out=dbg["act1d"][:], in_=act1[:])
            nc.sync.dma_start(out=dbg["act2d"][:], in_=act2[:])
            nc.sync.dma_start(out=dbg["act3d"][:], in_=act3[:])
            nc.sync.dma_start(out=dbg["act4d"][:], in_=act4[:])
            nc.sync.dma_start(out=dbg["h4d"][:], in_=h4g[0][:])

    nc.compile()
    return nc


# --------------------------------------------------------------------------
# Host-side preparation
# --------------------------------------------------------------------------

def _pack_w_dr(ws_t):
    """[Fin, Fout] {-1,+1} -> [Mt, 128, C, 2, 128] fp8 DoubleRow layout.

    wdr[mt, ki, c, ko, mi] = ws_t[256*c + 128*ko + ki, 128*mt + mi]
    """
    fin, fout = ws_t.shape
    C, Mt = fin // 256, fout // 128
    w = ws_t.reshape(C, 2, 128, Mt, 128).transpose(3, 2, 0, 1, 4)
    return np.ascontiguousarray(w).astype(NP_FP8)


def prepare_consts(inputs):
    """Fold sign(w), BN, bias and the 0/1-activation rowsum correction.

    The device computes, per layer, a_dev = [mmA~ >= thr] where
    mmA~ = W~sign @ a_dev_prev over {0,1} activations. Negative BN scales
    (alpha <= 0) are handled exactly by tracking a per-neuron flip bit
    (a_true = 1 - a_dev) that folds into the *next* layer's weight signs:
    with s~ = s * (1-2*flip_in), mm_full = 2*(s~ @ a_dev) - rowsum(s~)
    holds for any flip pattern. Thresholds use integer snapping (mmA is
    always an integer), making the device comparison tie-free/exact.
    """
    consts = {}
    flip_in = np.zeros(IND)  # input layer: a_dev = [x >= 0] = ste_sign, exact
    for i in (1, 2, 3, 4):
        w = np.asarray(inputs[f"w{i}"]).astype(np.float64)
        b = np.asarray(inputs[f"b{i}"]).astype(np.float64)
        g = np.asarray(inputs[f"g{i}"]).astype(np.float64)
        be = np.asarray(inputs[f"be{i}"]).astype(np.float64)
        m = np.asarray(inputs[f"m{i}"]).astype(np.float64)
        v = np.asarray(inputs[f"v{i}"]).astype(np.float64)
        ws = np.where(w >= 0, 1.0, -1.0) * (1.0 - 2.0 * flip_in)  # [fo, fi]
        rowsum = ws.sum(axis=1)                                   # [fo]
        alpha = g / np.sqrt(v + EPS)
        if i < 4:
            # BN(mm_full + b) >= 0 with mm_full = 2*mmA - rowsum:
            #   alpha > 0:  a_true = [mmA >= u],  u = (m-b-be/a+rowsum)/2
            #   alpha < 0:  a_true = [mmA <= u] = 1 - [mmA >= floor(u)+1]
            #   alpha == 0: BN = be, constant sign
            u = (m - b - be / alpha_safe(alpha) + rowsum) / 2.0
            pos = alpha > 0
            thr = np.where(pos, np.ceil(u), np.floor(u) + 1.0)
            zero = alpha == 0
            if zero.any():
                # constant: a_true = [be >= 0]; force a_dev accordingly
                thr = np.where(zero & (be >= 0), -1e30, thr)
                thr = np.where(zero & (be < 0), 1e30, thr)
                pos = pos | zero
            flip_in = (~pos).astype(np.float64)
            consts.setdefault("_thrs", []).append(
                thr.reshape(32, 128).T.astype(np.float32))
            consts[f"w{i}dr"] = _pack_w_dr(ws.T)
        else:
            # logits = mmA*(2*alpha) + ((b - m - rowsum)*alpha + be).
            # c4 affine replicated at the four column-group partition bases
            # (batch slice g computes at PSUM/SBUF partitions [32g, 32g+16)).
            scale = 2.0 * alpha
            beta = (b - m - rowsum) * alpha + be
            c4 = np.zeros((128, 2), np.float32)
            for g4 in range(4):
                c4[32 * g4:32 * g4 + 10, 0] = scale.astype(np.float32)
                c4[32 * g4:32 * g4 + 10, 1] = beta.astype(np.float32)
            consts["c4"] = c4
            ws_t_pad = np.zeros((HID, 16), np.float64)
            ws_t_pad[:, :10] = ws.T
            # w4dr[ki, kc, m] = ws_t_pad[128*kc + ki, m]  (no DoubleRow)
            w4 = ws_t_pad.reshape(32, 128, 16).transpose(1, 0, 2)
            consts["w4dr"] = np.ascontiguousarray(w4).astype(NP_FP8)
    consts["thrs"] = np.ascontiguousarray(
        np.stack(consts.pop("_thrs"), axis=1))  # [128, 3, 32]
    return consts


def alpha_safe(a):
    return np.where(a == 0, 1.0, a)


_PROG_CACHE = {}


def _get_program(bc=BC):
    if bc not in _PROG_CACHE:
        _PROG_CACHE[bc] = build_program(bc)
    return _PROG_CACHE[bc]


def kernel(**inputs):
    global LAST_RESULTS
    x = np.asarray(inputs["x"], np.float32)
    assert x.shape == (B, 784)
    consts = prepare_consts(inputs)
    # Binarize on the host: [x >= 0] in {0,1} is exactly ste_sign's predicate
    # (-0.0 >= 0 is True), stored fp8 in the device's [128, 3, 2, B] k-chunk
    # layout (k = 256*c + 128*ko + p).
    a = (x[:, :IND] >= 0).astype(NP_FP8)                       # [B, 768]
    a1_full = np.ascontiguousarray(
        a.T.reshape(3, 2, 128, B).transpose(2, 0, 1, 3))       # [128,3,2,B]

    nc = _get_program(BC)
    in_maps = []
    for c in range(N_CORES):
        m = {"a1": np.ascontiguousarray(a1_full[:, :, :, c * BC:(c + 1) * BC])}
        m.update(consts)
        in_maps.append(m)

    res = run_bass_kernel_spmd(
        nc, in_maps, core_ids=list(range(N_CORES)), trace=TRACE,
        **TRACE_KWARGS)
    LAST_RESULTS = res
    # device out is [128, BT, 10] partition-major; restore batch order
    outs = [np.ascontiguousarray(r["out"].transpose(1, 0, 2).reshape(BC, OUT))
            for r in res.results]
    return np.concatenate(outs, axis=0)


# revision 39
# speedup vs baseline: 1.0024x; 1.0024x over previous
"""Binarized-MLP (BinaryNet) forward on 8 Trainium2 NeuronCores.

Reference computation (per nn_FC_large):
    h = sign(x[:, :768]) @ sign(w1).T + b1 ; BN1 ; -> sign
    h = sign(h) @ sign(w2).T + b2         ; BN2 ; -> sign
    h = sign(h) @ sign(w3).T + b3         ; BN3 ; -> sign
    h = sign(h) @ sign(w4).T + b4         ; BN4 ; log_softmax

Strategy (data parallel, batch 16384 -> 2048 rows/core):
  * All matmul operands are exactly representable in fp8: weights are
    binarized host-side to {-1,+1}; activations are kept as a in {0,1}
    (a = [pre-act >= 0]) and the identity
        sign_mm = 2*(Wsign @ a) - rowsum(Wsign)
    folds rowsum into per-neuron thresholds, so each layer's epilogue is a
    single DVE is_ge producing the next layer's {0,1} fp8 activations.
  * Matmuls run in fp8e4 with perf_mode=DoubleRow (K=256 per instruction),
    activations stored feature-major [F, B] in SBUF across the whole net.
    The PE stream is the bottleneck (~216 ns per 512-col DR matmul, ~97% of
    the DoubleRow streaming roofline), so everything else hides under it.
  * Layer-1 activations are binarized on the HOST ([x >= 0] is exact in any
    dtype) and DMA'd straight into SBUF as fp8 {0,1}: halves the input DMA
    and removes the on-device binarize from the critical path. L1 starts
    ~4.5us into the kernel (vs ~20us with on-device bf16 binarize).
  * BatchNorm (eval) + bias fold into thresholds (layers 1-3) / an affine
    (layer 4). Layer-4 logits are PE-transposed to batch-major (interleaved
    with the L4 matmul groups) into a single PSUM bank, and log_softmax runs
    as 7 whole-tile ops (reduce_max / broadcast-sub / exp / reduce-add / ln /
    broadcast-sub) instead of 16 per-batch-tile chains.
  * The ACT table set `natural_log_exp_and_others` (identity+exp+ln) is
    preloaded at kernel start so no table swap lands on the critical tail.
  * Accumulation is exact: products are in {-1,0,1}, sums are integers
    well inside fp32, so the binary pipeline is bit-exact w.r.t. the
    reference up to threshold ties (probability ~0 with random BN stats).

Everything is hardcoded for x:[16384,784], layers 768->4096->4096->4096->10.
"""

import numpy as np
import ml_dtypes
from contextlib import ExitStack

import concourse.mybir as mybir
import concourse.tile as tile
from concourse import bacc
from concourse.bass_utils import run_bass_kernel_spmd
from concourse.masks import make_identity

FP32 = mybir.dt.float32
FP8 = mybir.dt.float8e4
NP_FP8 = ml_dtypes.float8_e4m3

EPS = 1e-5
B, IND, HID, OUT = 16384, 768, 4096, 10
N_CORES = 8
BC = B // N_CORES  # 2048 batch rows per core

# Knobs (test.py may flip TRACE before calling kernel()).
TRACE = False
TRACE_KWARGS = {}
LAST_RESULTS = None  # BassKernelResults of the most recent run

# act_info.json set 6 = natural_log_exp_and_others: {ln, exp, copy, identity}
ACT_SET_LN_EXP = 6


# --------------------------------------------------------------------------
# Device program
# --------------------------------------------------------------------------

def _layer_fwd(nc, wpool, psum_pool, act_in, C, wdr, thr_sb, act_out, Mt, bc,
               dma_engine=None, first_wt=None, pads=None):
    """One binarized layer: act_out = [W_fp8dr.T @ act_in >= thr] in {0,1} fp8.

    act_in : SBUF AP [128, C, 2, bc] fp8 ({0,1})
    wdr    : DRAM [Mt, 128, C, 2, 128] fp8 ({-1,+1})
    thr_sb : SBUF [128, Mt] fp32
    act_out: SBUF AP [128, Mt//2, 2, bc] fp8
    """
    NT = bc // 512
    DR = mybir.MatmulPerfMode.DoubleRow
    dma_engine = dma_engine or nc.sync
    first_wt = first_wt or {}
    for mt in range(Mt):
        if mt in first_wt:
            wt = first_wt[mt]
        else:
            wt = wpool.tile([128, C, 2, 128], FP8, tag="w")
            dma_engine.dma_start(out=wt[:], in_=wdr[mt])
        pss = [psum_pool.tile([128, 512], FP32, tag="psum", name=f"ps{mt}_{n}")
               for n in range(NT)]
        for c in range(C):
            for n in range(NT):
                nc.tensor.matmul(
                    pss[n][:],
                    lhsT=wt[:, c, :, :],
                    rhs=act_in[:, c, :, 512 * n:512 * (n + 1)],
                    start=(c == 0),
                    stop=(c == C - 1),
                    perf_mode=DR,
                )
            if pads and (mt, c) in pads:
                pads[(mt, c)]()
        for n in range(NT):
            nc.vector.tensor_scalar(
                out=act_out[:, mt // 2, mt % 2, 512 * n:512 * (n + 1)],
                in0=pss[n][:],
                scalar1=thr_sb[:, mt:mt + 1],
                scalar2=None,
                op0=mybir.AluOpType.is_ge,
            )


def build_program(bc=BC, dump_acts=False):
    """Build the per-core Bass/Tile program (SPMD; identical on all cores)."""
    NT = bc // 512
    BT = bc // 128
    DR = mybir.MatmulPerfMode.DoubleRow

    nc = bacc.Bacc(None, target_bir_lowering=False, debug=False)
    dbg = {}
    if dump_acts:
        for nm in ("act1d", "act2d", "act3d", "act4d"):
            cdim = 3 if nm == "act1d" else 16
            dbg[nm] = nc.dram_tensor(
                nm, [128, cdim, 2, bc], FP8, kind="ExternalOutput")
        dbg["h4d"] = nc.dram_tensor("h4d", [128, 512], FP32,
                                    kind="ExternalOutput")

    a1 = nc.dram_tensor("a1", [128, 3, 2, bc], FP8, kind="ExternalInput")
    w1 = nc.dram_tensor("w1dr", [32, 128, 3, 2, 128], FP8, kind="ExternalInput")
    w2 = nc.dram_tensor("w2dr", [32, 128, 16, 2, 128], FP8, kind="ExternalInput")
    w3 = nc.dram_tensor("w3dr", [32, 128, 16, 2, 128], FP8, kind="ExternalInput")
    w4 = nc.dram_tensor("w4dr", [128, 32, 16], FP8, kind="ExternalInput")
    thrs = nc.dram_tensor("thrs", [128, 3, 32], FP32, kind="ExternalInput")
    c4 = nc.dram_tensor("c4", [128, 2], FP32, kind="ExternalInput")
    out = nc.dram_tensor("out", [128, bc // 128, OUT], FP32,
                         kind="ExternalOutput")

    with tile.TileContext(nc) as tc, ExitStack() as ctx:
        consts = ctx.enter_context(tc.tile_pool(name="consts", bufs=1))
        a1pool = ctx.enter_context(tc.tile_pool(name="a1pool", bufs=1))
        apool = ctx.enter_context(
            tc.tile_pool(name="apool", bufs=3 if dump_acts else 2))
        wpool = ctx.enter_context(tc.tile_pool(name="wpool", bufs=4))
        smpool = ctx.enter_context(tc.tile_pool(name="smpool", bufs=3))
        psum_pool = ctx.enter_context(
            tc.tile_pool(name="psum", bufs=8, space="PSUM"))

        # Preload the exp+ln+identity ACT table set (runs during the ~7us
        # framework preamble, overlapping the DMA queue) so neither the L4
        # affine nor the softmax tail pays a ~2.7us table swap.
        nc.scalar.add_instruction(mybir.InstLoadActFuncSet(
            name=nc.get_next_instruction_name(), ins=[], outs=[],
            act_func_set_id=ACT_SET_LN_EXP))

        thrs_sb = consts.tile([128, 3, 32], FP32, tag="thrs")
        c4_sb = consts.tile([128, 2], FP32, tag="c4")
        w4_sb = consts.tile([128, 32, 16], FP8, tag="w4")
        ident = consts.tile([128, 128], FP32, tag="ident")
        h4 = consts.tile([128, 512], FP32, tag="h4")
        out_sb = consts.tile([128, BT, OUT], FP32, tag="outsb")
        thr1_sb = thrs_sb[:, 0, :]
        thr2_sb = thrs_sb[:, 1, :]
        thr3_sb = thrs_sb[:, 2, :]

        # ---- layer-1 activations: host-binarized {0,1} fp8. The first
        # k-chunk and the first L1 weight tile gate the start of the MM
        # stream, so they go FIRST on the sync queue (4KB/768B lines) with
        # everything else (small-line const DMAs, remaining chunks) behind
        # or on the scalar queue, keeping the DMA engines clear for them.
        # The early DMA burst is delivery-bandwidth-bound (~200-250 GB/s
        # effective across the two HWDGE queues), so everything is issued in
        # strict NEED order, with each act1 k-chunk split across both queues
        # so it lands at aggregate bandwidth: c0 -> w1[0] -> c1 -> c2 ->
        # w1[1] -> w1[2]. Later w1 tiles stream on sync far ahead of their
        # ~2.6us/tile consumption.
        act1 = a1pool.tile([128, 3, 2, bc], FP8, tag="act1")
        w1t = {}

        def _w1t(mt, eng):
            w1t[mt] = wpool.tile([128, 3, 2, 128], FP8, tag="w",
                                 name=f"w1t{mt}")
            eng.dma_start(out=w1t[mt][:], in_=w1[mt])

        nc.sync.dma_start(out=act1[:, 0], in_=a1[:, 0])
        _w1t(0, nc.sync)
        nc.sync.dma_start(out=act1[:, 1], in_=a1[:, 1])
        _w1t(1, nc.sync)
        _w1t(2, nc.sync)
        nc.sync.dma_start(out=act1[:, 2], in_=a1[:, 2])

        # thrs on the scalar ring (thr1 first needed ~20us in); c4/w4 are
        # only needed by layer 4 (~990us) and go at the very back.
        nc.scalar.dma_start(out=thrs_sb[:], in_=thrs[:])

        # PE warm-up: short (N=128) garbage DR matmuls (memset operands,
        # never-read psum) bridge the ~3.5us between the earliest possible
        # PE activity (~7.9us, post-preamble) and act1-c0/w1t0 landing
        # (~10.5us) at ~107ns granularity, keeping the HAM busy window
        # counting continuously so the clock un-gates ~3.4us after the
        # first warmup, right as layer 1 opens.
        warm = consts.tile([128, 2, 128], FP8, tag="warm")
        nc.gpsimd.memset(warm[:], 0.0)
        wps = psum_pool.tile([128, 128], FP32, tag="psum", name="warmps")
        for _ in range(34):
            nc.tensor.matmul(
                wps[:], lhsT=warm[:], rhs=warm[:],
                start=True, stop=True, perf_mode=DR)
        make_identity(nc, ident[:])

        def _pad(n_mms):
            # garbage matmuls bridging a known data stall so the PE never
            # idles long enough for the HAM clock gate to re-throttle
            def f():
                for _ in range(n_mms):
                    nc.tensor.matmul(
                        wps[:], lhsT=warm[:], rhs=warm[:],
                        start=True, stop=True, perf_mode=DR)
            return f

        # ---- layers 1-3 ----
        # a1-c2 lands ~2us after layer 1 first consumes it (delivery-bound);
        # pad the (mt0, c1->c2) boundary so the clock stays un-gated.
        act2 = apool.tile([128, 16, 2, bc], FP8, tag="actbig")
        _layer_fwd(nc, wpool, psum_pool, act1, 3, w1, thr1_sb, act2, 32, bc,
                   dma_engine=nc.scalar, first_wt=w1t,
                   pads={(0, 1): _pad(24)})

        # L4 consts: c4 must ride a HWDGE queue (a tiny engine-executed
        # DMACopy on the ACT queue would alias its completion into the ACT
        # op-count semaphore and falsely serialize the DVE affines behind
        # the ACT affines); w4 rides scalar behind the last L1 weight tiles.
        nc.sync.dma_start(out=c4_sb[:], in_=c4[:])
        nc.scalar.dma_start(out=w4_sb[:], in_=w4[:])


        act3 = apool.tile([128, 16, 2, bc], FP8, tag="actbig")
        _layer_fwd(nc, wpool, psum_pool, act2, 16, w2, thr2_sb, act3, 32, bc)
        act4 = apool.tile([128, 16, 2, bc], FP8, tag="actbig")
        _layer_fwd(nc, wpool, psum_pool, act3, 16, w3, thr3_sb, act4, 32, bc)

        # ---- layer 4: logits (M padded 10->16), affine folds BN+rowsum.
        # M=16 would waste 7/8 of the PE array in the normal layout, so the
        # four 512-column batch slices run CONCURRENTLY on the four 32-wide
        # PE column groups (plain fp8, no DoubleRow -- DR and column tiling
        # are mutually exclusive on the XBUS budget): slice g accumulates
        # K=4096 as 32 k-chunks into PSUM partitions [32g, 32g+16). The
        # whole GEMM spans ~32 matmul slots instead of 64 DR slots.
        # Affines (scalar engine) land the logits in SBUF at the same
        # partition bases; transposes lift each 128-batch tile to
        # batch-major [128, 10] in one PSUM bank for the softmax chain.
        tps = psum_pool.tile([128, BT * OUT], FP32, tag="psum", name="tps")
        ps4 = psum_pool.tile([128, 512], FP32, tag="psum", name="ps4")
        for kc in range(32):
            for g in range(NT):
                nc.tensor.matmul(
                    ps4[32 * g:32 * g + 16, :],
                    lhsT=w4_sb[:, kc, :],
                    rhs=act4[:, kc // 2, kc % 2, 512 * g:512 * (g + 1)],
                    start=(kc == 0),
                    stop=(kc == 31),
                    tile_position=(0, 32 * g),
                )
        # Per-slice affines on the scalar engine (the four slices occupy
        # disjoint partition ranges of the one PSUM bank).
        for g in range(NT):
            nc.scalar.activation(
                h4[32 * g:32 * g + 16, :], ps4[32 * g:32 * g + 16, :],
                mybir.ActivationFunctionType.Identity,
                bias=c4_sb[32 * g:32 * g + 16, 1:2],
                scale=c4_sb[32 * g:32 * g + 16, 0:1])
        for g in range(NT):
            for b in range(4):
                bt = 4 * g + b
                nc.tensor.transpose(
                    tps[:, OUT * bt:OUT * (bt + 1)],
                    h4[32 * g:32 * g + OUT, 128 * b:128 * (b + 1)],
                    ident[32 * g:32 * g + OUT, 32 * g:32 * g + OUT],
                    tile_position=(32 * g, 0))

        # ---- log_softmax, batched over all 16 batch-tiles at once ----
        tps3 = tps[:].rearrange("p (b o) -> p b o", b=BT)
        mx = smpool.tile([128, BT], FP32, tag="mx", bufs=1)
        nc.vector.reduce_max(mx[:], tps3, axis=mybir.AxisListType.X)
        sh = smpool.tile([128, BT, OUT], FP32, tag="sh", bufs=1)
        nc.vector.scalar_tensor_tensor(
            out=sh[:], in0=tps3, scalar=0.0,
            in1=mx[:].unsqueeze(2).broadcast_to([128, BT, OUT]),
            op0=mybir.AluOpType.add, op1=mybir.AluOpType.subtract)
        ex = smpool.tile([128, BT, OUT], FP32, tag="ex", bufs=1)
        nc.scalar.activation(ex[:], sh[:], mybir.ActivationFunctionType.Exp)
        se = smpool.tile([128, BT], FP32, tag="se", bufs=1)
        nc.vector.reduce_sum(se[:], ex[:], axis=mybir.AxisListType.X)
        ls = smpool.tile([128, BT], FP32, tag="ls", bufs=1)
        nc.scalar.activation(ls[:], se[:], mybir.ActivationFunctionType.Ln)
        nc.vector.scalar_tensor_tensor(
            out=out_sb[:], in0=sh[:], scalar=0.0,
            in1=ls[:].unsqueeze(2).broadcast_to([128, BT, OUT]),
            op0=mybir.AluOpType.add, op1=mybir.AluOpType.subtract)

        # out dram is [128, BT, OUT] (partition-major, fully contiguous DMA);
        # the host reassembles batch order with a free transpose.
        nc.sync.dma_start(out=out[:], in_=out_sb[:])

        if dump_acts:
            nc.sync.dma_start(out=dbg["act1d"][:], in_=act1[:])
            nc.sync.dma_start(out=dbg["act2d"][:], in_=act2[:])
            nc.sync.dma_start(out=dbg["act3d"][:], in_=act3[:])
            nc.sync.dma_start(out=dbg["act4d"][:], in_=act4[:])
            nc.sync.dma_start(out=dbg["h4d"][:], in_=h4[:])

    nc.compile()
    return nc


# --------------------------------------------------------------------------
# Host-side preparation
# --------------------------------------------------------------------------

def _pack_w_dr(ws_t):
    """[Fin, Fout] {-1,+1} -> [Mt, 128, C, 2, 128] fp8 DoubleRow layout.

    wdr[mt, ki, c, ko, mi] = ws_t[256*c + 128*ko + ki, 128*mt + mi]
    """
    fin, fout = ws_t.shape
    C, Mt = fin // 256, fout // 128
    w = ws_t.reshape(C, 2, 128, Mt, 128).transpose(3, 2, 0, 1, 4)
    return np.ascontiguousarray(w).astype(NP_FP8)


def prepare_consts(inputs):
    """Fold sign(w), BN, bias and the 0/1-activation rowsum correction.

    The device computes, per layer, a_dev = [mmA~ >= thr] where
    mmA~ = W~sign @ a_dev_prev over {0,1} activations. Negative BN scales
    (alpha <= 0) are handled exactly by tracking a per-neuron flip bit
    (a_true = 1 - a_dev) that folds into the *next* layer's weight signs:
    with s~ = s * (1-2*flip_in), mm_full = 2*(s~ @ a_dev) - rowsum(s~)
    holds for any flip pattern. Thresholds use integer snapping (mmA is
    always an integer), making the device comparison tie-free/exact.
    """
    consts = {}
    flip_in = np.zeros(IND)  # input layer: a_dev = [x >= 0] = ste_sign, exact
    for i in (1, 2, 3, 4):
        w = np.asarray(inputs[f"w{i}"]).astype(np.float64)
        b = np.asarray(inputs[f"b{i}"]).astype(np.float64)
        g = np.asarray(inputs[f"g{i}"]).astype(np.float64)
        be = np.asarray(inputs[f"be{i}"]).astype(np.float64)
        m = np.asarray(inputs[f"m{i}"]).astype(np.float64)
        v = np.asarray(inputs[f"v{i}"]).astype(np.float64)
        ws = np.where(w >= 0, 1.0, -1.0) * (1.0 - 2.0 * flip_in)  # [fo, fi]
        rowsum = ws.sum(axis=1)                                   # [fo]
        alpha = g / np.sqrt(v + EPS)
        if i < 4:
            # BN(mm_full + b) >= 0 with mm_full = 2*mmA - rowsum:
            #   alpha > 0:  a_true = [mmA >= u],  u = (m-b-be/a+rowsum)/2
            #   alpha < 0:  a_true = [mmA <= u] = 1 - [mmA >= floor(u)+1]
            #   alpha == 0: BN = be, constant sign
            u = (m - b - be / alpha_safe(alpha) + rowsum) / 2.0
            pos = alpha > 0
            thr = np.where(pos, np.ceil(u), np.floor(u) + 1.0)
            zero = alpha == 0
            if zero.any():
                # constant: a_true = [be >= 0]; force a_dev accordingly
                thr = np.where(zero & (be >= 0), -1e30, thr)
                thr = np.where(zero & (be < 0), 1e30, thr)
                pos = pos | zero
            flip_in = (~pos).astype(np.float64)
            consts.setdefault("_thrs", []).append(
                thr.reshape(32, 128).T.astype(np.float32))
            consts[f"w{i}dr"] = _pack_w_dr(ws.T)
        else:
            # logits = mmA*(2*alpha) + ((b - m - rowsum)*alpha + be).
            # c4 affine replicated at the four column-group partition bases
            # (batch slice g computes at PSUM/SBUF partitions [32g, 32g+16)).
            scale = 2.0 * alpha
            beta = (b - m - rowsum) * alpha + be
            c4 = np.zeros((128, 2), np.float32)
            for g4 in range(4):
                c4[32 * g4:32 * g4 + 10, 0] = scale.astype(np.float32)
                c4[32 * g4:32 * g4 + 10, 1] = beta.astype(np.float32)
            consts["c4"] = c4
            ws_t_pad = np.zeros((HID, 16), np.float64)
            ws_t_pad[:, :10] = ws.T
            # w4dr[ki, kc, m] = ws_t_pad[128*kc + ki, m]  (no DoubleRow)
            w4 = ws_t_pad.reshape(32, 128, 16).transpose(1, 0, 2)
            consts["w4dr"] = np.ascontiguousarray(w4).astype(NP_FP8)
    consts["thrs"] = np.ascontiguousarray(
        np.stack(consts.pop("_thrs"), axis=1))  # [128, 3, 32]
    return consts


def alpha_safe(a):
    return np.where(a == 0, 1.0, a)


_PROG_CACHE = {}


def _get_program(bc=BC):
    if bc not in _PROG_CACHE:
        _PROG_CACHE[bc] = build_program(bc)
    return _PROG_CACHE[bc]


def kernel(**inputs):
    global LAST_RESULTS
    x = np.asarray(inputs["x"], np.float32)
    assert x.shape == (B, 784)
    consts = prepare_consts(inputs)
    # Binarize on the host: [x >= 0] in {0,1} is exactly ste_sign's predicate
    # (-0.0 >= 0 is True), stored fp8 in the device's [128, 3, 2, B] k-chunk
    # layout (k = 256*c + 128*ko + p).
    a = (x[:, :IND] >= 0).astype(NP_FP8)                       # [B, 768]
    a1_full = np.ascontiguousarray(
        a.T.reshape(3, 2, 128, B).transpose(2, 0, 1, 3))       # [128,3,2,B]

    nc = _get_program(BC)
    in_maps = []
    for c in range(N_CORES):
        m = {"a1": np.ascontiguousarray(a1_full[:, :, :, c * BC:(c + 1) * BC])}
        m.update(consts)
        in_maps.append(m)

    res = run_bass_kernel_spmd(
        nc, in_maps, core_ids=list(range(N_CORES)), trace=TRACE,
        **TRACE_KWARGS)
    LAST_RESULTS = res
    # device out is [128, BT, 10] partition-major; restore batch order
    outs = [np.ascontiguousarray(r["out"].transpose(1, 0, 2).reshape(BC, OUT))
            for r in res.results]
    return np.concatenate(outs, axis=0)


# revision 45
# speedup vs baseline: 1.0028x; 1.0004x over previous
"""Binarized-MLP (BinaryNet) forward on 8 Trainium2 NeuronCores.

Reference computation (per nn_FC_large):
    h = sign(x[:, :768]) @ sign(w1).T + b1 ; BN1 ; -> sign
    h = sign(h) @ sign(w2).T + b2         ; BN2 ; -> sign
    h = sign(h) @ sign(w3).T + b3         ; BN3 ; -> sign
    h = sign(h) @ sign(w4).T + b4         ; BN4 ; log_softmax

Strategy (data parallel, batch 16384 -> 2048 rows/core):
  * All matmul operands are exactly representable in fp8: weights are
    binarized host-side to {-1,+1}; activations are kept as a in {0,1}
    (a = [pre-act >= 0]) and the identity
        sign_mm = 2*(Wsign @ a) - rowsum(Wsign)
    folds rowsum into per-neuron thresholds, so each layer's epilogue is a
    single DVE is_ge producing the next layer's {0,1} fp8 activations.
  * Matmuls run in fp8e4 with perf_mode=DoubleRow (K=256 per instruction),
    activations stored feature-major [F, B] in SBUF across the whole net.
    The PE stream is the bottleneck (~216 ns per 512-col DR matmul, ~97% of
    the DoubleRow streaming roofline), so everything else hides under it.
  * Layer-1 activations are binarized on the HOST ([x >= 0] is exact in any
    dtype) and DMA'd straight into SBUF as fp8 {0,1}: halves the input DMA
    and removes the on-device binarize from the critical path. L1 starts
    ~4.5us into the kernel (vs ~20us with on-device bf16 binarize).
  * BatchNorm (eval) + bias fold into thresholds (layers 1-3) / an affine
    (layer 4). Layer-4 logits are PE-transposed to batch-major (interleaved
    with the L4 matmul groups) into a single PSUM bank, and log_softmax runs
    as 7 whole-tile ops (reduce_max / broadcast-sub / exp / reduce-add / ln /
    broadcast-sub) instead of 16 per-batch-tile chains.
  * The ACT table set `natural_log_exp_and_others` (identity+exp+ln) is
    preloaded at kernel start so no table swap lands on the critical tail.
  * Accumulation is exact: products are in {-1,0,1}, sums are integers
    well inside fp32, so the binary pipeline is bit-exact w.r.t. the
    reference up to threshold ties (probability ~0 with random BN stats).

Everything is hardcoded for x:[16384,784], layers 768->4096->4096->4096->10.
"""

import numpy as np
import ml_dtypes
from contextlib import ExitStack

import concourse.mybir as mybir
import concourse.tile as tile
from concourse import bacc
from concourse.bass_utils import run_bass_kernel_spmd
from concourse.masks import make_identity

FP32 = mybir.dt.float32
FP8 = mybir.dt.float8e4
NP_FP8 = ml_dtypes.float8_e4m3

EPS = 1e-5
B, IND, HID, OUT = 16384, 768, 4096, 10
N_CORES = 8
BC = B // N_CORES  # 2048 batch rows per core

# Knobs (test.py may flip TRACE before calling kernel()).
TRACE = False
TRACE_KWARGS = {}
LAST_RESULTS = None  # BassKernelResults of the most recent run

# act_info.json set 6 = natural_log_exp_and_others: {ln, exp, copy, identity}
ACT_SET_LN_EXP = 6


# --------------------------------------------------------------------------
# Device program
# --------------------------------------------------------------------------

def _layer_fwd(nc, wpool, psum_pool, act_in, C, wdr, thr_sb, act_out, Mt, bc,
               dma_engine=None, first_wt=None, pads=None):
    """One binarized layer: act_out = [W_fp8dr.T @ act_in >= thr] in {0,1} fp8.

    act_in : SBUF AP [128, C, 2, bc] fp8 ({0,1})
    wdr    : DRAM [Mt, 128, C, 2, 128] fp8 ({-1,+1})
    thr_sb : SBUF [128, Mt] fp32
    act_out: SBUF AP [128, Mt//2, 2, bc] fp8
    """
    NT = bc // 512
    DR = mybir.MatmulPerfMode.DoubleRow
    dma_engine = dma_engine or nc.sync
    first_wt = first_wt or {}
    for mt in range(Mt):
        if mt in first_wt:
            wt = first_wt[mt]
        else:
            wt = wpool.tile([128, C, 2, 128], FP8, tag="w")
            dma_engine.dma_start(out=wt[:], in_=wdr[mt])
        pss = [psum_pool.tile([128, 512], FP32, tag="psum", name=f"ps{mt}_{n}")
               for n in range(NT)]
        for c in range(C):
            for n in range(NT):
                nc.tensor.matmul(
                    pss[n][:],
                    lhsT=wt[:, c, :, :],
                    rhs=act_in[:, c, :, 512 * n:512 * (n + 1)],
                    start=(c == 0),
                    stop=(c == C - 1),
                    perf_mode=DR,
                )
            if pads and (mt, c) in pads:
                pads[(mt, c)]()
        for n in range(NT):
            nc.vector.tensor_scalar(
                out=act_out[:, mt // 2, mt % 2, 512 * n:512 * (n + 1)],
                in0=pss[n][:],
                scalar1=thr_sb[:, mt:mt + 1],
                scalar2=None,
                op0=mybir.AluOpType.is_ge,
            )


def build_program(bc=BC, dump_acts=False):
    """Build the per-core Bass/Tile program (SPMD; identical on all cores)."""
    NT = bc // 512
    BT = bc // 128
    DR = mybir.MatmulPerfMode.DoubleRow

    nc = bacc.Bacc(None, target_bir_lowering=False, debug=False)
    dbg = {}
    if dump_acts:
        for nm in ("act1d", "act2d", "act3d", "act4d"):
            cdim = 3 if nm == "act1d" else 16
            dbg[nm] = nc.dram_tensor(
                nm, [128, cdim, 2, bc], FP8, kind="ExternalOutput")
        dbg["h4d"] = nc.dram_tensor("h4d", [128, 512], FP32,
                                    kind="ExternalOutput")

    a1 = nc.dram_tensor("a1", [128, 3, 2, bc], FP8, kind="ExternalInput")
    w1 = nc.dram_tensor("w1dr", [32, 128, 3, 2, 128], FP8, kind="ExternalInput")
    w2 = nc.dram_tensor("w2dr", [32, 128, 16, 2, 128], FP8, kind="ExternalInput")
    w3 = nc.dram_tensor("w3dr", [32, 128, 16, 2, 128], FP8, kind="ExternalInput")
    w4 = nc.dram_tensor("w4dr", [128, 32, 16], FP8, kind="ExternalInput")
    thrs = nc.dram_tensor("thrs", [128, 3, 32], FP32, kind="ExternalInput")
    c4 = nc.dram_tensor("c4", [128, 2], FP32, kind="ExternalInput")
    out = nc.dram_tensor("out", [128, bc // 128, OUT], FP32,
                         kind="ExternalOutput")

    with tile.TileContext(nc) as tc, ExitStack() as ctx:
        consts = ctx.enter_context(tc.tile_pool(name="consts", bufs=1))
        a1pool = ctx.enter_context(tc.tile_pool(name="a1pool", bufs=1))
        apool = ctx.enter_context(
            tc.tile_pool(name="apool", bufs=3 if dump_acts else 2))
        wpool = ctx.enter_context(tc.tile_pool(name="wpool", bufs=4))
        smpool = ctx.enter_context(tc.tile_pool(name="smpool", bufs=3))
        psum_pool = ctx.enter_context(
            tc.tile_pool(name="psum", bufs=8, space="PSUM"))

        # Preload the exp+ln+identity ACT table set (runs during the ~7us
        # framework preamble, overlapping the DMA queue) so neither the L4
        # affine nor the softmax tail pays a ~2.7us table swap.
        nc.scalar.add_instruction(mybir.InstLoadActFuncSet(
            name=nc.get_next_instruction_name(), ins=[], outs=[],
            act_func_set_id=ACT_SET_LN_EXP))

        thrs_sb = consts.tile([128, 3, 32], FP32, tag="thrs")
        c4_sb = consts.tile([128, 2], FP32, tag="c4")
        w4_sb = consts.tile([128, 32, 16], FP8, tag="w4")
        ident = consts.tile([128, 128], FP32, tag="ident")
        h4 = consts.tile([128, 512], FP32, tag="h4")
        out_sb = consts.tile([128, BT, OUT], FP32, tag="outsb")
        thr1_sb = thrs_sb[:, 0, :]
        thr2_sb = thrs_sb[:, 1, :]
        thr3_sb = thrs_sb[:, 2, :]

        # ---- layer-1 activations: host-binarized {0,1} fp8. The first
        # k-chunk and the first L1 weight tile gate the start of the MM
        # stream, so they go FIRST on the sync queue (4KB/768B lines) with
        # everything else (small-line const DMAs, remaining chunks) behind
        # or on the scalar queue, keeping the DMA engines clear for them.
        # The early DMA burst is delivery-bandwidth-bound (~200-250 GB/s
        # effective across the two HWDGE queues), so everything is issued in
        # strict NEED order, with each act1 k-chunk split across both queues
        # so it lands at aggregate bandwidth: c0 -> w1[0] -> c1 -> c2 ->
        # w1[1] -> w1[2]. Later w1 tiles stream on sync far ahead of their
        # ~2.6us/tile consumption.
        act1 = a1pool.tile([128, 3, 2, bc], FP8, tag="act1")
        w1t = {}

        def _w1t(mt, eng):
            w1t[mt] = wpool.tile([128, 3, 2, 128], FP8, tag="w",
                                 name=f"w1t{mt}")
            eng.dma_start(out=w1t[mt][:], in_=w1[mt])

        nc.sync.dma_start(out=act1[:, 0], in_=a1[:, 0])
        _w1t(0, nc.sync)
        nc.sync.dma_start(out=act1[:, 1], in_=a1[:, 1])
        _w1t(1, nc.sync)
        _w1t(2, nc.sync)
        nc.sync.dma_start(out=act1[:, 2], in_=a1[:, 2])

        # thrs on the scalar ring (thr1 first needed ~20us in); c4/w4 are
        # only needed by layer 4 (~990us) and go at the very back.
        nc.scalar.dma_start(out=thrs_sb[:], in_=thrs[:])

        # PE warm-up: short (N=128) garbage DR matmuls (memset operands,
        # never-read psum) bridge the ~3.5us between the earliest possible
        # PE activity (~7.9us, post-preamble) and act1-c0/w1t0 landing
        # (~10.5us) at ~107ns granularity, keeping the HAM busy window
        # counting continuously so the clock un-gates ~3.4us after the
        # first warmup, right as layer 1 opens.
        warm = consts.tile([128, 2, 128], FP8, tag="warm")
        nc.gpsimd.memset(warm[:], 0.0)
        wps = psum_pool.tile([128, 128], FP32, tag="psum", name="warmps")
        for _ in range(34):
            nc.tensor.matmul(
                wps[:], lhsT=warm[:], rhs=warm[:],
                start=True, stop=True, perf_mode=DR)
        make_identity(nc, ident[:])

        def _pad(n_mms):
            # garbage matmuls bridging a known data stall so the PE never
            # idles long enough for the HAM clock gate to re-throttle
            def f():
                for _ in range(n_mms):
                    nc.tensor.matmul(
                        wps[:], lhsT=warm[:], rhs=warm[:],
                        start=True, stop=True, perf_mode=DR)
            return f

        # ---- layers 1-3 ----
        # a1-c2 lands ~2us after layer 1 first consumes it (delivery-bound);
        # pad the (mt0, c1->c2) boundary so the clock stays un-gated.
        act2 = apool.tile([128, 16, 2, bc], FP8, tag="actbig")
        _layer_fwd(nc, wpool, psum_pool, act1, 3, w1, thr1_sb, act2, 32, bc,
                   dma_engine=nc.scalar, first_wt=w1t,
                   pads={(0, 1): _pad(24)})

        # L4 consts: c4 must ride a HWDGE queue (a tiny engine-executed
        # DMACopy on the ACT queue would alias its completion into the ACT
        # op-count semaphore and falsely serialize the DVE affines behind
        # the ACT affines); w4 rides scalar behind the last L1 weight tiles.
        nc.sync.dma_start(out=c4_sb[:], in_=c4[:])
        nc.scalar.dma_start(out=w4_sb[:], in_=w4[:])


        act3 = apool.tile([128, 16, 2, bc], FP8, tag="actbig")
        _layer_fwd(nc, wpool, psum_pool, act2, 16, w2, thr2_sb, act3, 32, bc)
        act4 = apool.tile([128, 16, 2, bc], FP8, tag="actbig")
        _layer_fwd(nc, wpool, psum_pool, act3, 16, w3, thr3_sb, act4, 32, bc)

        # ---- layer 4: logits (M padded 10->16), affine folds BN+rowsum.
        # M=16 would waste 7/8 of the PE array in the normal layout, so the
        # four 512-column batch slices run CONCURRENTLY on the four 32-wide
        # PE column groups (plain fp8, no DoubleRow -- DR and column tiling
        # are mutually exclusive on the XBUS budget): slice g accumulates
        # K=4096 as 32 k-chunks into PSUM partitions [32g, 32g+16). The
        # whole GEMM spans ~32 matmul slots instead of 64 DR slots.
        # Affines (scalar engine) land the logits in SBUF at the same
        # partition bases; transposes lift each 128-batch tile to
        # batch-major [128, 10] in one PSUM bank for the softmax chain.
        tps = psum_pool.tile([128, BT * OUT], FP32, tag="psum", name="tps")
        ps4 = psum_pool.tile([128, 512], FP32, tag="psum", name="ps4")
        for kc in range(32):
            for g in range(NT):
                nc.tensor.matmul(
                    ps4[32 * g:32 * g + 16, :],
                    lhsT=w4_sb[:, kc, :],
                    rhs=act4[:, kc // 2, kc % 2, 512 * g:512 * (g + 1)],
                    start=(kc == 0),
                    stop=(kc == 31),
                    tile_position=(0, 32 * g),
                )
        # Per-slice affines on the scalar engine (the four slices occupy
        # disjoint partition ranges of the one PSUM bank).
        for g in range(NT):
            nc.scalar.activation(
                h4[32 * g:32 * g + 16, :], ps4[32 * g:32 * g + 16, :],
                mybir.ActivationFunctionType.Identity,
                bias=c4_sb[32 * g:32 * g + 16, 1:2],
                scale=c4_sb[32 * g:32 * g + 16, 0:1])
        for g in range(NT):
            for b in range(4):
                bt = 4 * g + b
                nc.tensor.transpose(
                    tps[:, OUT * bt:OUT * (bt + 1)],
                    h4[32 * g:32 * g + OUT, 128 * b:128 * (b + 1)],
                    ident[32 * g:32 * g + OUT, 32 * g:32 * g + OUT],
                    tile_position=(32 * g, 0))

        # ---- log_softmax, batched over all 16 batch-tiles at once ----
        tps3 = tps[:].rearrange("p (b o) -> p b o", b=BT)
        mx = smpool.tile([128, BT], FP32, tag="mx", bufs=1)
        nc.vector.reduce_max(mx[:], tps3, axis=mybir.AxisListType.X)
        sh = smpool.tile([128, BT, OUT], FP32, tag="sh", bufs=1)
        nc.vector.scalar_tensor_tensor(
            out=sh[:], in0=tps3, scalar=0.0,
            in1=mx[:].unsqueeze(2).broadcast_to([128, BT, OUT]),
            op0=mybir.AluOpType.add, op1=mybir.AluOpType.subtract)
        ex = smpool.tile([128, BT, OUT], FP32, tag="ex", bufs=1)
        nc.scalar.activation(ex[:], sh[:], mybir.ActivationFunctionType.Exp)
        se = smpool.tile([128, BT], FP32, tag="se", bufs=1)
        nc.vector.reduce_sum(se[:], ex[:], axis=mybir.AxisListType.X)
        ls = smpool.tile([128, BT], FP32, tag="ls", bufs=1)
        nc.scalar.activation(ls[:], se[:], mybir.ActivationFunctionType.Ln)
        nc.vector.scalar_tensor_tensor(
            out=out_sb[:], in0=sh[:], scalar=0.0,
            in1=ls[:].unsqueeze(2).broadcast_to([128, BT, OUT]),
            op0=mybir.AluOpType.add, op1=mybir.AluOpType.subtract)

        # out dram is [128, BT, OUT] (partition-major, fully contiguous DMA);
        # the host reassembles batch order with a free transpose.
        nc.sync.dma_start(out=out[:], in_=out_sb[:])

        if dump_acts:
            nc.sync.dma_start(out=dbg["act1d"][:], in_=act1[:])
            nc.sync.dma_start(out=dbg["act2d"][:], in_=act2[:])
            nc.sync.dma_start(out=dbg["act3d"][:], in_=act3[:])
            nc.sync.dma_start(out=dbg["act4d"][:], in_=act4[:])
            nc.sync.dma_start(out=dbg["h4d"][:], in_=h4[:])

    nc.compile()
    return nc


# --------------------------------------------------------------------------
# Host-side preparation
# --------------------------------------------------------------------------

def _pack_w_dr(ws_t):
    """[Fin, Fout] {-1,+1} -> [Mt, 128, C, 2, 128] fp8 DoubleRow layout.

    wdr[mt, ki, c, ko, mi] = ws_t[256*c + 128*ko + ki, 128*mt + mi]
    """
    fin, fout = ws_t.shape
    C, Mt = fin // 256, fout // 128
    w = ws_t.reshape(C, 2, 128, Mt, 128).transpose(3, 2, 0, 1, 4)
    return np.ascontiguousarray(w).astype(NP_FP8)


def prepare_consts(inputs):
    """Fold sign(w), BN, bias and the 0/1-activation rowsum correction.

    The device computes, per layer, a_dev = [mmA~ >= thr] where
    mmA~ = W~sign @ a_dev_prev over {0,1} activations. Negative BN scales
    (alpha <= 0) are handled exactly by tracking a per-neuron flip bit
    (a_true = 1 - a_dev) that folds into the *next* layer's weight signs:
    with s~ = s * (1-2*flip_in), mm_full = 2*(s~ @ a_dev) - rowsum(s~)
    holds for any flip pattern. Thresholds use integer snapping (mmA is
    always an integer), making the device comparison tie-free/exact.
    """
    consts = {}
    flip_in = np.zeros(IND)  # input layer: a_dev = [x >= 0] = ste_sign, exact
    for i in (1, 2, 3, 4):
        w = np.asarray(inputs[f"w{i}"]).astype(np.float64)
        b = np.asarray(inputs[f"b{i}"]).astype(np.float64)
        g = np.asarray(inputs[f"g{i}"]).astype(np.float64)
        be = np.asarray(inputs[f"be{i}"]).astype(np.float64)
        m = np.asarray(inputs[f"m{i}"]).astype(np.float64)
        v = np.asarray(inputs[f"v{i}"]).astype(np.float64)
        ws = np.where(w >= 0, 1.0, -1.0) * (1.0 - 2.0 * flip_in)  # [fo, fi]
        rowsum = ws.sum(axis=1)                                   # [fo]
        alpha = g / np.sqrt(v + EPS)
        if i < 4:
            # BN(mm_full + b) >= 0 with mm_full = 2*mmA - rowsum:
            #   alpha > 0:  a_true = [mmA >= u],  u = (m-b-be/a+rowsum)/2
            #   alpha < 0:  a_true = [mmA <= u] = 1 - [mmA >= floor(u)+1]
            #   alpha == 0: BN = be, constant sign
            u = (m - b - be / alpha_safe(alpha) + rowsum) / 2.0
            pos = alpha > 0
            thr = np.where(pos, np.ceil(u), np.floor(u) + 1.0)
            zero = alpha == 0
            if zero.any():
                # constant: a_true = [be >= 0]; force a_dev accordingly
                thr = np.where(zero & (be >= 0), -1e30, thr)
                thr = np.where(zero & (be < 0), 1e30, thr)
                pos = pos | zero
            flip_in = (~pos).astype(np.float64)
            consts.setdefault("_thrs", []).append(
                thr.reshape(32, 128).T.astype(np.float32))
            consts[f"w{i}dr"] = _pack_w_dr(ws.T)
        else:
            # logits = mmA*(2*alpha) + ((b - m - rowsum)*alpha + be).
            # c4 affine replicated at the four column-group partition bases
            # (batch slice g computes at PSUM/SBUF partitions [32g, 32g+16)).
            scale = 2.0 * alpha
            beta = (b - m - rowsum) * alpha + be
            c4 = np.zeros((128, 2), np.float32)
            for g4 in range(4):
                c4[32 * g4:32 * g4 + 10, 0] = scale.astype(np.float32)
                c4[32 * g4:32 * g4 + 10, 1] = beta.astype(np.float32)
            consts["c4"] = c4
            ws_t_pad = np.zeros((HID, 16), np.float64)
            ws_t_pad[:, :10] = ws.T
            # w4dr[ki, kc, m] = ws_t_pad[128*kc + ki, m]  (no DoubleRow)
            w4 = ws_t_pad.reshape(32, 128, 16).transpose(1, 0, 2)
            consts["w4dr"] = np.ascontiguousarray(w4).astype(NP_FP8)
    consts["thrs"] = np.ascontiguousarray(
        np.stack(consts.pop("_thrs"), axis=1))  # [128, 3, 32]
    return consts


def alpha_safe(a):
    return np.where(a == 0, 1.0, a)


_PROG_CACHE = {}


def _get_program(bc=BC):
    if bc not in _PROG_CACHE:
        _PROG_CACHE[bc] = build_program(bc)
    return _PROG_CACHE[bc]


def kernel(**inputs):
    global LAST_RESULTS
    x = np.asarray(inputs["x"], np.float32)
    assert x.shape == (B, 784)
    consts = prepare_consts(inputs)
    # Binarize on the host: [x >= 0] in {0,1} is exactly ste_sign's predicate
    # (-0.0 >= 0 is True), stored fp8 in the device's [128, 3, 2, B] k-chunk
    # layout (k = 256*c + 128*ko + p).
    a = (x[:, :IND] >= 0).astype(NP_FP8)                       # [B, 768]
    a1_full = np.ascontiguousarray(
        a.T.reshape(3, 2, 128, B).transpose(2, 0, 1, 3))       # [128,3,2,B]

    nc = _get_program(BC)
    in_maps = []
    for c in range(N_CORES):
        m = {"a1": np.ascontiguousarray(a1_full[:, :, :, c * BC:(c + 1) * BC])}
        m.update(consts)
        in_maps.append(m)

    res = run_bass_kernel_spmd(
        nc, in_maps, core_ids=list(range(N_CORES)), trace=TRACE,
        **TRACE_KWARGS)
    LAST_RESULTS = res
    # device out is [128, BT, 10] partition-major; restore batch order
    outs = [np.ascontiguousarray(r["out"].transpose(1, 0, 2).reshape(BC, OUT))
            for r in res.results]
    return np.concatenate(outs, axis=0)
